# revision 5
# baseline (speedup 1.0000x reference)
"""AdaptiveLabelPropagation Trainium2 kernel v5 (8 NeuronCores, SPMD).

Design (v5: src-sharded, scatter-free, gather-rate-optimized)
-------------------------------------------------------------
v2 was bottlenecked by SWDGE descriptor generation with poor queue
overlap (~4.8ns/idx effective).  A microbenchmark shows back-to-back
dma_gathers round-robined over the 4 SWDGE queues sustain ~2.5ns/idx.
v5 restructures for that rate and shrinks per-layer non-gather work:

* Edges sharded by *src* core; positions grouped by (dst segment, src
  window) exactly as v2; scatter is the one-hot matmul into a PSUM
  accumulator (no dma_scatter_add anywhere).
* Layer 0 fuses phase B: one 512B gather per edge from hncur_full
  (hn row ++ cur0 row) + one 256B local gather of hn[src]; cosine
  sims and layer-0 aggregation come out of the same gathered tiles.
* Layers 1-4 gather from a COMPACT cur table: rows of 32 bf16 (64B);
  one 256B gather element covers 4 consecutive nodes and a cheap DVE
  4-way masked select (mask prescaled by the edge weight) extracts
  w * cur[dst].  This shrinks the per-layer AllGather 4x (0.8MB in /
  6.4MB out per core) and needs no expansion pass.
* The layers' gather indices (node//4 per segment) are loaded into
  SBUF once and reused by all 4 layers; layer-0 indices stream per
  chunk as in v2.
* Every gather goes on SWDGE queue chunk%4 so all four queue pairs
  generate descriptors concurrently.
"""

import sys

if "/opt/trn_rl_repo" not in sys.path:
    sys.path.insert(0, "/opt/trn_rl_repo")

import numpy as np

import concourse.bacc as bacc
import concourse.tile as tile
from concourse import mybir
from concourse.bass_utils import run_bass_kernel_spmd

F32 = mybir.dt.float32
BF16 = mybir.dt.bfloat16
I16 = mybir.dt.int16

N, D, C, E = 100000, 128, 16, 1000000
NUM_LAYERS, ALPHA = 5, 0.5
EPS_COS, EPS_LN = 1e-8, 1e-5
NCORES = 8
SH = N // NCORES          # 12500 real rows per shard
SHP = 12544               # padded shard rows (98 * 128)
NW = SHP // 128           # 98 src windows per shard
NSEG = 8                  # dst segments
NT = NW                   # feature tiles in phase A
CHUNK0 = 2048             # layer-0 gather chunk (positions)
CHUNK = 4096              # layer 1-4 gather chunk (positions)


# ----------------------------------------------------------------- host prep


def _sigmoid(x):
    return 1.0 / (1.0 + np.exp(-np.float64(x)))


def preprocess(inputs):
    """Returns (in_maps, static_cfg). static_cfg = flattened gsz[seg][win]."""
    src = np.concatenate(
        [inputs["src_connect"], inputs["src_decorate"], inputs["src_next"]]
    ).astype(np.int64)
    dst = np.concatenate(
        [inputs["dst_connect"], inputs["dst_decorate"], inputs["dst_next"]]
    ).astype(np.int64)
    sig = np.concatenate(
        [
            np.full(E, _sigmoid(inputs["ew_connect"][0]), np.float32),
            np.full(E, _sigmoid(inputs["ew_decorate"][0]), np.float32),
            np.full(E, _sigmoid(inputs["ew_next"][0]), np.float32),
        ]
    )

    core = src // SH
    s_local = src - core * SH
    seg = dst // SH
    d_idx = (dst - seg * SH).astype(np.int64)  # [0, 12500)
    win = s_local // 128
    slot = (s_local % 128).astype(np.int16)

    cnt = np.zeros((NCORES, NSEG, NW), np.int64)
    np.add.at(cnt, (core, seg, win), 1)
    gsz = ((cnt.max(axis=0) + 127) // 128 * 128).astype(np.int64)  # [8, 98]
    assert (cnt.sum(axis=(0, 1)) > 0).all()
    off = np.zeros((NSEG, NW), np.int64)
    off.ravel()[1:] = np.cumsum(gsz.ravel())[:-1]
    NPOS = int(gsz.sum())

    # rank of each edge within its (core, seg, win) bucket, dst-sorted
    order = np.lexsort((dst, win, seg, core))
    key = ((core * NSEG + seg) * NW + win)[order]
    rs = np.r_[True, key[1:] != key[:-1]]
    rid = np.cumsum(rs) - 1
    fp = np.zeros(rid[-1] + 1, np.int64)
    fp[rid[rs]] = np.nonzero(rs)[0]
    within = np.empty(len(order), np.int64)
    within[order] = np.arange(len(order)) - fp[rid]

    pos = off[seg, win] + within  # core-local position

    idx_dst = np.zeros((NCORES, NPOS), np.int16)   # layer-0: node within seg
    idx_dst4 = np.zeros((NCORES, NPOS), np.int16)  # layers: node//4 within seg
    dm4 = np.zeros((NCORES, NPOS), np.int16)       # node%4 within seg
    idx_src = np.zeros((NCORES, NPOS), np.int16)
    slot_a = np.zeros((NCORES, NPOS), np.int16)
    scale = np.zeros((NCORES, NPOS), np.float32)
    idx_dst[core, pos] = d_idx.astype(np.int16)
    idx_dst4[core, pos] = (d_idx // 4).astype(np.int16)
    dm4[core, pos] = (d_idx % 4).astype(np.int16)
    idx_src[core, pos] = s_local.astype(np.int16)
    slot_a[core, pos] = slot
    scale[core, pos] = sig

    def wrap_idx(a):  # [NPOS] -> [128, NPOS//16] (16-wrap replicated 8x)
        w = a.reshape(-1, 16).T
        return np.ascontiguousarray(np.tile(w, (8, 1)))

    def poslay(a, dt):  # [NPOS] -> [128, NPOS//128] position layout
        return np.ascontiguousarray(a.reshape(-1, 128).T.astype(dt))

    feats = np.asarray(inputs["features"], np.float32)
    init = np.asarray(inputs["init_logits"], np.float32)
    W = np.asarray(inputs["W"], np.float32)
    b = np.asarray(inputs["b"], np.float32)
    gam = np.asarray(inputs["ln_gamma"], np.float32)
    bet = np.asarray(inputs["ln_beta"], np.float32)

    iota128 = np.tile(np.arange(128, dtype=np.float32)[None, :], (128, 1))
    iota4 = np.tile(np.arange(4, dtype=np.float32)[None, :], (128, 1))

    in_maps = []
    for c in range(NCORES):
        lo, hi = c * SH, (c + 1) * SH
        curinit = np.zeros((SHP, 128), np.float32)
        curinit[:SH, 0:C] = init[lo:hi]
        curinit[:, C] = 1.0
        featT = np.zeros((D, SHP), np.float32)
        featT[:, :SH] = feats[lo:hi].T
        ih = np.zeros((128, NW * C), np.float32)
        ihr = np.zeros((SHP, C), np.float32)
        ihr[:SH] = (1.0 - ALPHA) * init[lo:hi]
        # ih[p, w*C + c] = ihr[128*w + p, c]
        ih[:] = ihr.reshape(NW, 128, C).transpose(1, 0, 2).reshape(128, NW * C)
        in_maps.append(
            {
                "featT": featT,
                "wt": np.ascontiguousarray(W.T),
                "brow": np.ascontiguousarray(np.tile(b[None, :], (128, 1))),
                "grow": np.ascontiguousarray(np.tile(gam[None, :], (128, 1))),
                "berow": np.ascontiguousarray(np.tile(bet[None, :], (128, 1))),
                "iota": _to_bf16(iota128),
                "iota4": _to_bf16(iota4),
                "slotrow": _to_bf16(poslay(slot_a[c], np.float32)),
                "dm4row": _to_bf16(poslay(dm4[c], np.float32)),
                "scale": poslay(scale[c], np.float32),
                "idx_dst": wrap_idx(idx_dst[c]),
                "idx_dst4": wrap_idx(idx_dst4[c]),
                "idx_src": wrap_idx(idx_src[c]),
                "ih": ih,
                "curinit": _to_bf16(curinit),
            }
        )
    return in_maps, tuple(int(x) for x in gsz.ravel())


def _to_bf16(a):
    """Round-to-nearest-even f32 -> bf16, kept as ml_dtypes/np bfloat16."""
    import ml_dtypes

    return np.asarray(a, np.float32).astype(ml_dtypes.bfloat16)


# ------------------------------------------------------------------- builder


def build(nc, gsz_flat):
    gsz = np.asarray(gsz_flat, np.int64).reshape(NSEG, NW)
    off = np.zeros((NSEG, NW), np.int64)
    off.ravel()[1:] = np.cumsum(gsz.ravel())[:-1]
    NPOS = int(gsz.sum())
    NTIL = NPOS // 128

    # global tile t -> window
    tilewin = np.zeros(NTIL, np.int64)
    for s in range(NSEG):
        for w in range(NW):
            t0 = off[s, w] // 128
            tilewin[t0 : t0 + gsz[s, w] // 128] = w
    # PSUM `start=True` clears has_written bits for the WHOLE bank, so with
    # 32 windows sharing a bank the start flag must be issued exactly once
    # per (bank, layer) — on the first matmul touching that bank.  The
    # per-element has_written bits then give overwrite-on-first-touch /
    # accumulate-afterwards semantics for every window region in the bank.
    tilebank = tilewin // 32  # acc_ps bank of each tile (512 f32 per bank)
    first = np.zeros(NTIL, bool)
    last = np.zeros(NTIL, bool)
    for bk in range(4):
        ts = np.nonzero(tilebank == bk)[0]
        assert len(ts) > 0
        first[ts[0]] = True
        last[ts[-1]] = True

    # per-seg gather chunks (seg, pos_start, n_pos)
    def mkchunks(csz):
        chunks = []
        for s in range(NSEG):
            p0 = int(off[s, 0])
            send = p0 + int(gsz[s].sum())
            p = p0
            while p < send:
                n = min(csz, send - p)
                chunks.append((s, p, n))
                p += n
        return chunks

    chunks0 = mkchunks(CHUNK0)
    chunks = mkchunks(CHUNK)

    # ---- I/O
    featT = nc.dram_tensor("featT", [D, SHP], F32, kind="ExternalInput")
    wt = nc.dram_tensor("wt", [D, D], F32, kind="ExternalInput")
    brow = nc.dram_tensor("brow", [128, D], F32, kind="ExternalInput")
    grow = nc.dram_tensor("grow", [128, D], F32, kind="ExternalInput")
    berow = nc.dram_tensor("berow", [128, D], F32, kind="ExternalInput")
    iota_d = nc.dram_tensor("iota", [128, 128], BF16, kind="ExternalInput")
    iota4_d = nc.dram_tensor("iota4", [128, 4], BF16, kind="ExternalInput")
    slotrow_d = nc.dram_tensor("slotrow", [128, NTIL], BF16, kind="ExternalInput")
    dm4row_d = nc.dram_tensor("dm4row", [128, NTIL], BF16, kind="ExternalInput")
    scale_d = nc.dram_tensor("scale", [128, NTIL], F32, kind="ExternalInput")
    idx_dst = nc.dram_tensor("idx_dst", [128, NPOS // 16], I16, kind="ExternalInput")
    idx_dst4 = nc.dram_tensor("idx_dst4", [128, NPOS // 16], I16, kind="ExternalInput")
    idx_src = nc.dram_tensor("idx_src", [128, NPOS // 16], I16, kind="ExternalInput")
    ih_d = nc.dram_tensor("ih", [128, NW * C], F32, kind="ExternalInput")
    curinit = nc.dram_tensor("curinit", [SHP, 128], BF16, kind="ExternalInput")
    out = nc.dram_tensor("out", [SHP, C], F32, kind="ExternalOutput")

    # ---- internal DRAM
    # hncur rows pack [hn (256B) | cur0 (256B)] so ONE 512B gather per edge
    # serves both the phase-B similarity and the layer-0 aggregation.
    hn_c = nc.dram_tensor("hn_c", [SHP, D], BF16)
    hncur_loc = nc.dram_tensor("hncur_loc", [SHP, 256], BF16)
    hncur_full = nc.dram_tensor(
        "hncur_full", [NCORES * SHP, 256], BF16, addr_space="Shared"
    )
    # compact cur for layers >= 1: 32 bf16 per node (cols 0:16 used)
    curloc = [nc.dram_tensor(f"curloc{i}", [SHP, 32], BF16) for i in range(2)]
    curfull = [
        nc.dram_tensor(f"curfull{i}", [NCORES * SHP, 32], BF16, addr_space="Shared")
        for i in range(2)
    ]

    rg = [list(range(NCORES))]
    tc = nc._tc

    # =================================================== phase A: features
    with tc.tile_pool(name="pa", bufs=2) as pa, tc.tile_pool(
        name="pa1", bufs=1
    ) as pa1, tc.tile_pool(name="pap", bufs=2, space="PSUM") as pap:
        ft = pa1.tile([128, SHP], F32)
        nc.sync.dma_start(out=ft[:], in_=featT[:])
        wts = pa1.tile([128, D], F32)
        nc.sync.dma_start(out=wts[:], in_=wt[:])
        brs = pa1.tile([128, D], F32)
        nc.sync.dma_start(out=brs[:], in_=brow[:])
        grs = pa1.tile([128, D], F32)
        nc.sync.dma_start(out=grs[:], in_=grow[:])
        bes = pa1.tile([128, D], F32)
        nc.sync.dma_start(out=bes[:], in_=berow[:])
        epsl = pa1.tile([128, 1], F32)
        nc.vector.memset(epsl[:], EPS_LN)

        for t in range(NT):
            ps = pap.tile([128, D], F32)
            nc.tensor.matmul(
                out=ps[:],
                lhsT=ft[:, t * 128 : (t + 1) * 128],
                rhs=wts[:],
                start=True,
                stop=True,
            )
            h = pa.tile([128, D], F32)
            nc.vector.tensor_tensor(
                out=h[:], in0=ps[:], in1=brs[:], op=mybir.AluOpType.add
            )
            stats = pa.tile([128, 6], F32)
            nc.vector.bn_stats(out=stats[:], in_=h[:])
            mv = pa.tile([128, 2], F32)
            nc.vector.bn_aggr(out=mv[:], in_=stats[:])
            std = pa.tile([128, 1], F32)
            nc.scalar.activation(
                out=std[:],
                in_=mv[:, 1:2],
                func=mybir.ActivationFunctionType.Sqrt,
                bias=epsl[:],
                scale=1.0,
            )
            rstd = pa.tile([128, 1], F32)
            nc.vector.reciprocal(out=rstd[:], in_=std[:])
            hc = pa.tile([128, D], F32)
            nc.vector.scalar_tensor_tensor(
                out=hc[:],
                in0=h[:],
                scalar=mv[:, 0:1],
                in1=rstd[:].to_broadcast([128, D]),
                op0=mybir.AluOpType.subtract,
                op1=mybir.AluOpType.mult,
            )
            hg = pa.tile([128, D], F32)
            nc.vector.tensor_tensor(
                out=hg[:], in0=hc[:], in1=grs[:], op=mybir.AluOpType.mult
            )
            hb = pa.tile([128, D], F32)
            nc.vector.tensor_tensor(
                out=hb[:], in0=hg[:], in1=bes[:], op=mybir.AluOpType.add
            )
            hr = pa.tile([128, D], F32)
            nc.vector.tensor_scalar(
                out=hr[:],
                in0=hb[:],
                scalar1=0.0,
                scalar2=None,
                op0=mybir.AluOpType.max,
            )
            sq = pa.tile([128, D], F32)
            nc.vector.tensor_tensor(
                out=sq[:], in0=hr[:], in1=hr[:], op=mybir.AluOpType.mult
            )
            ssum = pa.tile([128, 1], F32)
            nc.vector.tensor_reduce(
                out=ssum[:], in_=sq[:], axis=mybir.AxisListType.X,
                op=mybir.AluOpType.add,
            )
            snrm = pa.tile([128, 1], F32)
            nc.scalar.activation(
                out=snrm[:],
                in_=ssum[:],
                func=mybir.ActivationFunctionType.Sqrt,
            )
            scl = pa.tile([128, 1], F32)
            nc.vector.tensor_scalar(
                out=scl[:],
                in0=snrm[:],
                scalar1=EPS_COS,
                scalar2=None,
                op0=mybir.AluOpType.max,
            )
            rcl = pa.tile([128, 1], F32)
            nc.vector.reciprocal(out=rcl[:], in_=scl[:])
            hnf = pa.tile([128, D], BF16)
            nc.vector.tensor_scalar(
                out=hnf[:],
                in0=hr[:],
                scalar1=rcl[:],
                scalar2=None,
                op0=mybir.AluOpType.mult,
            )
            nc.sync.dma_start(
                out=hn_c[t * 128 : (t + 1) * 128, :], in_=hnf[:]
            )
            nc.sync.dma_start(
                out=hncur_loc[t * 128 : (t + 1) * 128, 0:128], in_=hnf[:]
            )

    nc.sync.dma_start(out=hncur_loc[:, 128:256], in_=curinit[:])
    nc.gpsimd.collective_compute(
        "AllGather",
        mybir.AluOpType.bypass,
        ins=[hncur_loc[:]],
        outs=[hncur_full[:]],
        replica_groups=rg,
    )

    # ------------------------------------------------- resident SBUF state
    pr_cm = tc.tile_pool(name="pr", bufs=1)
    pr = pr_cm.__enter__()
    w_sb = pr.tile([128, NTIL], BF16)
    slotrow = pr.tile([128, NTIL], BF16)
    nc.sync.dma_start(out=slotrow[:], in_=slotrow_d[:])
    dm4row = pr.tile([128, NTIL], BF16)
    nc.sync.dma_start(out=dm4row[:], in_=dm4row_d[:])
    iota = pr.tile([128, 128], BF16)
    nc.sync.dma_start(out=iota[:], in_=iota_d[:])
    iota4 = pr.tile([128, 4], BF16)
    nc.sync.dma_start(out=iota4[:], in_=iota4_d[:])
    ih_sb = pr.tile([128, NW * C], F32)
    nc.sync.dma_start(out=ih_sb[:], in_=ih_d[:])
    idx4_sb = pr.tile([128, NPOS // 16], I16)
    nc.sync.dma_start(out=idx4_sb[:], in_=idx_dst4[:])
    rdenom = pr.tile([128, NW], F32)
    curtile = pr.tile([128, NW, C], F32)

    # ============================== phase B + layers (B fused into layer 0)
    pcp_cm = tc.tile_pool(name="pcp", bufs=1, space="PSUM")
    pcp = pcp_cm.__enter__()
    acc_ps = pcp.tile([128, NW, C], F32)
    tw_ps = pcp.tile([128, NW], F32)

    # ------------------------------------------------ layer 0 (+ phase B)
    with tc.tile_pool(name="pg0", bufs=3) as pg, \
         tc.tile_pool(name="poh0", bufs=3) as poh, \
         tc.tile_pool(name="pv0", bufs=4) as pv, \
         tc.tile_pool(name="pb0", bufs=1) as pb1:
        scale_sb = pb1.tile([128, NTIL], F32)
        nc.sync.dma_start(out=scale_sb[:], in_=scale_d[:])
        for ci, (s, p0, np_) in enumerate(chunks0):
            til = np_ // 128
            t0 = p0 // 128
            icol = p0 // 16
            qn = ci % 4
            idd = pg.tile([128, CHUNK0 // 16], I16, tag="idd")
            nc.sync.dma_start(
                out=idd[:, : np_ // 16],
                in_=idx_dst[:, icol : icol + np_ // 16],
            )
            G = pg.tile([128, CHUNK0 // 128, 256], BF16, tag="G")
            nc.gpsimd.dma_gather(
                out_ap=G[:, :til],
                in_ap=hncur_full[s * SHP : (s + 1) * SHP, :],
                idxs_ap=idd[:, : np_ // 16],
                num_idxs=np_,
                num_idxs_reg=np_,
                elem_size=256,
                single_packet=False,
                queue_num=qn,
            )
            # fused phase B: gather hn[src], w = sig*cos(hn_s, hn_d)
            ids = pg.tile([128, CHUNK0 // 16], I16, tag="ids")
            nc.sync.dma_start(
                out=ids[:, : np_ // 16],
                in_=idx_src[:, icol : icol + np_ // 16],
            )
            gs = pg.tile([128, CHUNK0 // 128, D], BF16, tag="gs")
            nc.gpsimd.dma_gather(
                out_ap=gs[:, :til],
                in_ap=hn_c[:],
                idxs_ap=ids[:, : np_ // 16],
                num_idxs=np_,
                num_idxs_reg=np_,
                elem_size=D,
                single_packet=False,
                queue_num=qn,
            )
            nc.vector.tensor_tensor(
                out=gs[:, :til], in0=gs[:, :til], in1=G[:, :til, 0:D],
                op=mybir.AluOpType.mult,
            )
            sim = pv.tile([128, CHUNK0 // 128, 1], F32, tag="sim")
            nc.vector.tensor_reduce(
                out=sim[:, :til], in_=gs[:, :til],
                axis=mybir.AxisListType.X, op=mybir.AluOpType.add,
            )
            nc.vector.tensor_tensor(
                out=w_sb[:, t0 : t0 + til],
                in0=sim[:, :til, 0],
                in1=scale_sb[:, t0 : t0 + til],
                op=mybir.AluOpType.mult,
            )
            oh = poh.tile([128, CHUNK0 // 128, 128], BF16, tag="oh")
            nc.vector.tensor_tensor(
                out=oh[:, :til],
                in0=iota[:].unsqueeze(1).to_broadcast([128, til, 128]),
                in1=slotrow[:, t0 : t0 + til]
                .unsqueeze(2)
                .to_broadcast([128, til, 128]),
                op=mybir.AluOpType.is_equal,
            )
            # V = w * G on the 17 used columns only
            V = pv.tile([128, CHUNK0 // 128, C + 1], BF16, tag="V")
            nc.vector.tensor_tensor(
                out=V[:, :til],
                in0=G[:, :til, 128 : 128 + C + 1],
                in1=w_sb[:, t0 : t0 + til]
                .unsqueeze(2)
                .to_broadcast([128, til, C + 1]),
                op=mybir.AluOpType.mult,
            )
            for ti in range(til):
                t = t0 + ti
                w = int(tilewin[t])
                nc.tensor.matmul(
                    out=acc_ps[:, w, :],
                    lhsT=oh[:, ti, :],
                    rhs=V[:, ti, 0:C],
                    start=bool(first[t]),
                    stop=bool(last[t]),
                    skip_group_check=True,
                )
                # tw_ps has its own bank: start on tile 0 only
                nc.tensor.matmul(
                    out=tw_ps[:, w : w + 1],
                    lhsT=oh[:, ti, :],
                    rhs=V[:, ti, C : C + 1],
                    start=(t == 0),
                    stop=(t == NTIL - 1),
                    skip_group_check=True,
                )

    # layer-0 epilogue: denominators + cur_1
    with tc.tile_pool(name="pe0", bufs=1) as pe:
        mask = pe.tile([128, NW], F32)
        nc.vector.tensor_scalar(
            out=mask[:], in0=tw_ps[:], scalar1=0.0, scalar2=None,
            op0=mybir.AluOpType.is_gt,
        )
        ones = pe.tile([128, NW], F32)
        nc.vector.memset(ones[:], 1.0)
        md = pe.tile([128, NW], F32)
        nc.vector.scalar_tensor_tensor(
            out=md[:], in0=mask[:], scalar=-1.0, in1=ones[:],
            op0=mybir.AluOpType.mult, op1=mybir.AluOpType.add,
        )
        tm = pe.tile([128, NW], F32)
        nc.vector.tensor_tensor(
            out=tm[:], in0=tw_ps[:], in1=mask[:], op=mybir.AluOpType.mult
        )
        dn = pe.tile([128, NW], F32)
        nc.vector.tensor_tensor(
            out=dn[:], in0=tm[:], in1=md[:], op=mybir.AluOpType.add
        )
        rec = pe.tile([128, NW], F32)
        nc.vector.reciprocal(out=rec[:], in_=dn[:])
        nc.vector.tensor_scalar(
            out=rdenom[:], in0=rec[:], scalar1=ALPHA, scalar2=None,
            op0=mybir.AluOpType.mult,
        )
        nc.vector.tensor_tensor(
            out=curtile[:],
            in0=acc_ps[:],
            in1=rdenom[:].unsqueeze(2).to_broadcast([128, NW, C]),
            op=mybir.AluOpType.mult,
        )
        nc.vector.tensor_tensor(
            out=curtile[:],
            in0=curtile[:],
            in1=ih_sb[:].rearrange("p (t c) -> p t c", c=C),
            op=mybir.AluOpType.add,
        )
        cb = pe.tile([128, NW, 32], BF16)
        nc.vector.memset(cb[:], 0.0)
        nc.vector.tensor_scalar(
            out=cb[:, :, 0:C], in0=curtile[:], scalar1=0.0, scalar2=None,
            op0=mybir.AluOpType.add,
        )
        nc.sync.dma_start(
            out=curloc[1 % 2][:].rearrange("(t p) c -> p t c", p=128),
            in_=cb[:],
        )
    nc.gpsimd.collective_compute(
        "AllGather",
        mybir.AluOpType.bypass,
        ins=[curloc[1][:]],
        outs=[curfull[1][:]],
        replica_groups=rg,
    )

    # --------------------------------------------------- layers 1..4
    for layer in range(1, NUM_LAYERS):
        src_tab = curfull[layer % 2]
        tab4 = src_tab[:].rearrange("(a b) c -> a (b c)", b=4)  # [8*3136, 128]
        with tc.tile_pool(name=f"pg{layer}", bufs=6) as pg, \
             tc.tile_pool(name=f"poh{layer}", bufs=3) as poh, \
             tc.tile_pool(name=f"pv{layer}", bufs=3) as pv:
            for ci, (s, p0, np_) in enumerate(chunks):
                til = np_ // 128
                t0 = p0 // 128
                icol = p0 // 16
                G = pg.tile([128, CHUNK // 128, 128], BF16, tag="G")
                nc.gpsimd.dma_gather(
                    out_ap=G[:, :til],
                    in_ap=tab4[s * (SHP // 4) : (s + 1) * (SHP // 4), :],
                    idxs_ap=idx4_sb[:, icol : icol + np_ // 16],
                    num_idxs=np_,
                    num_idxs_reg=np_,
                    elem_size=128,
                    single_packet=False,
                    queue_num=ci % 4,
                )
                # mask_w[p, ti, j] = w * (dm4 == j)
                mw = pv.tile([128, CHUNK // 128, 4], BF16, tag="mw")
                nc.vector.tensor_tensor(
                    out=mw[:, :til],
                    in0=iota4[:].unsqueeze(1).to_broadcast([128, til, 4]),
                    in1=dm4row[:, t0 : t0 + til]
                    .unsqueeze(2)
                    .to_broadcast([128, til, 4]),
                    op=mybir.AluOpType.is_equal,
                )
                nc.vector.tensor_tensor(
                    out=mw[:, :til],
                    in0=mw[:, :til],
                    in1=w_sb[:, t0 : t0 + til]
                    .unsqueeze(2)
                    .to_broadcast([128, til, 4]),
                    op=mybir.AluOpType.mult,
                )
                # V = sum_j mw[.., j] * G4[.., j, 0:C]
                Gv = G[:, :, :].rearrange("p t (j c) -> p t j c", j=4)
                V = pv.tile([128, CHUNK // 128, C], BF16, tag="V")
                acc0 = pv.tile([128, CHUNK // 128, C], BF16, tag="acc0")
                nc.vector.tensor_tensor(
                    out=V[:, :til],
                    in0=Gv[:, :til, 0, 0:C],
                    in1=mw[:, :til, 0:1].to_broadcast([128, til, C]),
                    op=mybir.AluOpType.mult,
                )
                for j in range(1, 4):
                    nc.vector.tensor_tensor(
                        out=acc0[:, :til],
                        in0=Gv[:, :til, j, 0:C],
                        in1=mw[:, :til, j : j + 1].to_broadcast([128, til, C]),
                        op=mybir.AluOpType.mult,
                    )
                    nc.vector.tensor_tensor(
                        out=V[:, :til],
                        in0=V[:, :til],
                        in1=acc0[:, :til],
                        op=mybir.AluOpType.add,
                    )
                oh = poh.tile([128, CHUNK // 128, 128], BF16, tag="oh")
                nc.vector.tensor_tensor(
                    out=oh[:, :til],
                    in0=iota[:].unsqueeze(1).to_broadcast([128, til, 128]),
                    in1=slotrow[:, t0 : t0 + til]
                    .unsqueeze(2)
                    .to_broadcast([128, til, 128]),
                    op=mybir.AluOpType.is_equal,
                )
                for ti in range(til):
                    t = t0 + ti
                    w = int(tilewin[t])
                    nc.tensor.matmul(
                        out=acc_ps[:, w, :],
                        lhsT=oh[:, ti, :],
                        rhs=V[:, ti, :],
                        start=bool(first[t]),
                        stop=bool(last[t]),
                        skip_group_check=True,
                    )

        # ------------------------------------------------- layer epilogue
        with tc.tile_pool(name=f"pe{layer}", bufs=1) as pe:
            nc.vector.tensor_tensor(
                out=curtile[:],
                in0=acc_ps[:],
                in1=rdenom[:].unsqueeze(2).to_broadcast([128, NW, C]),
                op=mybir.AluOpType.mult,
            )
            nc.vector.tensor_tensor(
                out=curtile[:],
                in0=curtile[:],
                in1=ih_sb[:].rearrange("p (t c) -> p t c", c=C),
                op=mybir.AluOpType.add,
            )
            if layer < NUM_LAYERS - 1:
                cb = pe.tile([128, NW, 32], BF16)
                nc.vector.memset(cb[:], 0.0)
                nc.vector.tensor_scalar(
                    out=cb[:, :, 0:C], in0=curtile[:], scalar1=0.0,
                    scalar2=None, op0=mybir.AluOpType.add,
                )
                nc.sync.dma_start(
                    out=curloc[(layer + 1) % 2][:].rearrange(
                        "(t p) c -> p t c", p=128
                    ),
                    in_=cb[:],
                )
                nc.gpsimd.collective_compute(
                    "AllGather",
                    mybir.AluOpType.bypass,
                    ins=[curloc[(layer + 1) % 2][:]],
                    outs=[curfull[(layer + 1) % 2][:]],
                    replica_groups=rg,
                )
            else:
                nc.sync.dma_start(
                    out=out[:].rearrange("(t p) c -> p t c", p=128),
                    in_=curtile[:],
                )

    pcp_cm.__exit__(None, None, None)
    pr_cm.__exit__(None, None, None)


def fix_queue_nums(nc):
    """Align each custom Pool-DMA's SWDGE queue with its Tile-assigned
    DMASW sem lane (shadow-sem rule: one queue per sem)."""
    n = 0
    for inst in nc.inst_map.values():
        if type(inst).__name__ in ("InstDMAGatherAnt", "InstDMAScatterAddAnt"):
            p = getattr(inst, "bass_scheduled_proc", None)
            if p is not None and 11 <= p <= 18:
                inst.queue_num = (p - 11) % 4
                n += 1
    return n


# ------------------------------------------------------------------ runtime

_CACHE = {}


def _get_nc(cfg):
    key = tuple(cfg)
    if key not in _CACHE:
        nc = bacc.Bacc(
            "TRN2",
            target_bir_lowering=False,
            debug=False,
            enable_asserts=False,
            num_devices=NCORES,
            num_swdge_queues=4,
        )
        with tile.TileContext(nc) as tc:
            nc._tc = tc
            build(nc, cfg)
        fix_queue_nums(nc)
        nc.compile()
        _CACHE[key] = nc
    return _CACHE[key]


TRACE = False
LAST_RESULT = None


def _install_ntff_hook():
    """Provide antenv.axon_hooks (absent in this image) so that
    run_bass_kernel_spmd(trace=True) can capture NTFF profiles."""
    import types

    if "antenv.axon_hooks" in sys.modules:
        return
    import antenv

    mod = types.ModuleType("antenv.axon_hooks")
    mod._hook = None

    def set_axon_ntff_profile_hook(h):
        mod._hook = h

    def get_axon_ntff_profile_hook():
        return mod._hook

    mod.set_axon_ntff_profile_hook = set_axon_ntff_profile_hook
    mod.get_axon_ntff_profile_hook = get_axon_ntff_profile_hook
    sys.modules["antenv.axon_hooks"] = mod
    antenv.axon_hooks = mod
    try:
        from trn_agent_boot.trn_boot import _ntff_profile_via_ctypes

        h = _ntff_profile_via_ctypes("/opt/axon/libaxon_pjrt.so")
        if h is not None:
            set_axon_ntff_profile_hook(h)
    except Exception as e:  # degrade to no tracing
        print(f"ntff hook install failed: {e}", file=sys.stderr)


def kernel(**inputs):
    global LAST_RESULT
    if TRACE:
        _install_ntff_hook()
    in_maps, cfg = preprocess(inputs)
    nc = _get_nc(cfg)
    res = run_bass_kernel_spmd(
        nc, in_maps, core_ids=list(range(NCORES)), trace=TRACE
    )
    LAST_RESULT = res
    full = np.zeros((N, C), np.float32)
    for c in range(NCORES):
        full[c * SH : (c + 1) * SH] = res.results[c]["out"][:SH]
    return full


# revision 7
# speedup vs baseline: 1.0846x; 1.0846x over previous
"""AdaptiveLabelPropagation Trainium2 kernel v6 (8 NeuronCores, SPMD).

Design (v6: src-sharded, scatter-free, dispatch-minimized)
----------------------------------------------------------
v2/v5 analysis: the SWDGE gathers (Q7 descriptor generation, ~2.5ns/idx
when 4 queues run back-to-back) are throttled in-kernel by (a) the DVE
instruction DISPATCH rate (~1.6us per instruction, 9-10 per chunk in
v5) which paces the whole pipeline through the tile-pool ticks, (b)
DVE<->GpSimd SBUF port contention while DVE runs (+29% measured), and
(c) PE dispatch (4 instructions per tile in layer 0 due to separate
total-weight matmuls).

v6 keeps v2's proven structure (src-sharded edges, (seg,win) buckets,
one-hot matmul scatter, fused phase-B/layer-0 512B hncur gathers,
per-layer 256B cur gathers + full-row AllGather) and minimizes
per-chunk instruction counts:

* acc_ps is [128, 98, 32] f32: 32-f32 stride makes every window
  bank-aligned (16 windows per 2KB bank); V carries 17 columns
  (w*cur ++ w*1) so ONE matmul per tile accumulates both next_logits
  and total_w.  Layer 0 PE work halves; tw_ps is gone.
* Layers 1-4: exactly 2 DVE ops per 4096-edge chunk (one-hot build +
  V scale); layer 0: 5.  No 4-way select, no idx DMAs in layers
  (indices resident in SBUF, loaded after layer 0 releases its pools).
* Gathers round-robin the 4 SWDGE queues (plus the proc-aligned
  queue fixup), CHUNK=4096 everywhere.
"""

import sys

if "/opt/trn_rl_repo" not in sys.path:
    sys.path.insert(0, "/opt/trn_rl_repo")

import numpy as np

import concourse.bacc as bacc
import concourse.tile as tile
from concourse import mybir
from concourse.bass_utils import run_bass_kernel_spmd

F32 = mybir.dt.float32
BF16 = mybir.dt.bfloat16
I16 = mybir.dt.int16

N, D, C, E = 100000, 128, 16, 1000000
NUM_LAYERS, ALPHA = 5, 0.5
EPS_COS, EPS_LN = 1e-8, 1e-5
NCORES = 8
SH = N // NCORES          # 12500 real rows per shard
SHP = 12544               # padded shard rows (98 * 128)
NW = SHP // 128           # 98 src windows per shard
NSEG = 8                  # dst segments
NT = NW                   # feature tiles in phase A
CHUNK = 4096              # gather chunk (positions)
ACCW = 32                 # f32 stride of one window in acc_ps (bank-aligned)


# ----------------------------------------------------------------- host prep


def _sigmoid(x):
    return 1.0 / (1.0 + np.exp(-np.float64(x)))


def preprocess(inputs):
    """Returns (in_maps, static_cfg). static_cfg = flattened gsz[seg][win]."""
    src = np.concatenate(
        [inputs["src_connect"], inputs["src_decorate"], inputs["src_next"]]
    ).astype(np.int64)
    dst = np.concatenate(
        [inputs["dst_connect"], inputs["dst_decorate"], inputs["dst_next"]]
    ).astype(np.int64)
    sig = np.concatenate(
        [
            np.full(E, _sigmoid(inputs["ew_connect"][0]), np.float32),
            np.full(E, _sigmoid(inputs["ew_decorate"][0]), np.float32),
            np.full(E, _sigmoid(inputs["ew_next"][0]), np.float32),
        ]
    )

    core = src // SH
    s_local = src - core * SH
    seg = dst // SH
    d_idx = (dst - seg * SH).astype(np.int16)
    win = s_local // 128
    slot = (s_local % 128).astype(np.int16)

    cnt = np.zeros((NCORES, NSEG, NW), np.int64)
    np.add.at(cnt, (core, seg, win), 1)
    gsz = ((cnt.max(axis=0) + 127) // 128 * 128).astype(np.int64)  # [8, 98]
    assert (cnt.sum(axis=(0, 1)) > 0).all()
    off = np.zeros((NSEG, NW), np.int64)
    off.ravel()[1:] = np.cumsum(gsz.ravel())[:-1]
    NPOS = int(gsz.sum())

    # rank of each edge within its (core, seg, win) bucket, dst-sorted
    order = np.lexsort((dst, win, seg, core))
    key = ((core * NSEG + seg) * NW + win)[order]
    rs = np.r_[True, key[1:] != key[:-1]]
    rid = np.cumsum(rs) - 1
    fp = np.zeros(rid[-1] + 1, np.int64)
    fp[rid[rs]] = np.nonzero(rs)[0]
    within = np.empty(len(order), np.int64)
    within[order] = np.arange(len(order)) - fp[rid]

    pos = off[seg, win] + within  # core-local position

    idx_dst = np.zeros((NCORES, NPOS), np.int16)
    idx_src = np.zeros((NCORES, NPOS), np.int16)
    slot_a = np.zeros((NCORES, NPOS), np.int16)
    scale = np.zeros((NCORES, NPOS), np.float32)
    idx_dst[core, pos] = d_idx
    idx_src[core, pos] = s_local.astype(np.int16)
    slot_a[core, pos] = slot
    scale[core, pos] = sig

    def wrap_idx(a):  # [NPOS] -> [128, NPOS//16] (16-wrap replicated 8x)
        w = a.reshape(-1, 16).T
        return np.ascontiguousarray(np.tile(w, (8, 1)))

    def poslay(a, dt):  # [NPOS] -> [128, NPOS//128] position layout
        return np.ascontiguousarray(a.reshape(-1, 128).T.astype(dt))

    feats = np.asarray(inputs["features"], np.float32)
    init = np.asarray(inputs["init_logits"], np.float32)
    W = np.asarray(inputs["W"], np.float32)
    b = np.asarray(inputs["b"], np.float32)
    gam = np.asarray(inputs["ln_gamma"], np.float32)
    bet = np.asarray(inputs["ln_beta"], np.float32)

    iota128 = np.tile(np.arange(128, dtype=np.float32)[None, :], (128, 1))

    in_maps = []
    for c in range(NCORES):
        lo, hi = c * SH, (c + 1) * SH
        curinit = np.zeros((SHP, 128), np.float32)
        curinit[:SH, 0:C] = init[lo:hi]
        curinit[:, C] = 1.0
        featT = np.zeros((D, SHP), np.float32)
        featT[:, :SH] = feats[lo:hi].T
        ih = np.zeros((128, NW * C), np.float32)
        ihr = np.zeros((SHP, C), np.float32)
        ihr[:SH] = (1.0 - ALPHA) * init[lo:hi]
        # ih[p, w*C + c] = ihr[128*w + p, c]
        ih[:] = ihr.reshape(NW, 128, C).transpose(1, 0, 2).reshape(128, NW * C)
        in_maps.append(
            {
                "featT": featT,
                "wt": np.ascontiguousarray(W.T),
                "brow": np.ascontiguousarray(np.tile(b[None, :], (128, 1))),
                "grow": np.ascontiguousarray(np.tile(gam[None, :], (128, 1))),
                "berow": np.ascontiguousarray(np.tile(bet[None, :], (128, 1))),
                "iota": _to_bf16(iota128),
                "slotrow": _to_bf16(poslay(slot_a[c], np.float32)),
                "scale": poslay(scale[c], np.float32),
                "idx_dst": wrap_idx(idx_dst[c]),
                "idx_src": wrap_idx(idx_src[c]),
                "ih": ih,
                "curinit": _to_bf16(curinit),
            }
        )
    return in_maps, tuple(int(x) for x in gsz.ravel())


def _to_bf16(a):
    """Round-to-nearest-even f32 -> bf16, kept as ml_dtypes/np bfloat16."""
    import ml_dtypes

    return np.asarray(a, np.float32).astype(ml_dtypes.bfloat16)


# ------------------------------------------------------------------- builder


def build(nc, gsz_flat):
    gsz = np.asarray(gsz_flat, np.int64).reshape(NSEG, NW)
    off = np.zeros((NSEG, NW), np.int64)
    off.ravel()[1:] = np.cumsum(gsz.ravel())[:-1]
    NPOS = int(gsz.sum())
    NTIL = NPOS // 128

    # global tile t -> window
    tilewin = np.zeros(NTIL, np.int64)
    for s in range(NSEG):
        for w in range(NW):
            t0 = off[s, w] // 128
            tilewin[t0 : t0 + gsz[s, w] // 128] = w
    # acc_ps windows are 32 f32 wide -> 16 windows per 2KB PSUM bank.
    # `start=True` clears has_written for the whole bank, so issue it only
    # on the first matmul touching each bank (and `stop` on the last).
    NBANK = (NW + 15) // 16
    tilebank = tilewin // 16
    first = np.zeros(NTIL, bool)
    last = np.zeros(NTIL, bool)
    for bk in range(NBANK):
        ts = np.nonzero(tilebank == bk)[0]
        assert len(ts) > 0
        first[ts[0]] = True
        last[ts[-1]] = True

    # per-seg gather chunks (seg, pos_start, n_pos)
    chunks = []
    for s in range(NSEG):
        p0 = int(off[s, 0])
        send = p0 + int(gsz[s].sum())
        p = p0
        while p < send:
            n = min(CHUNK, send - p)
            chunks.append((s, p, n))
            p += n

    # ---- I/O
    featT = nc.dram_tensor("featT", [D, SHP], F32, kind="ExternalInput")
    wt = nc.dram_tensor("wt", [D, D], F32, kind="ExternalInput")
    brow = nc.dram_tensor("brow", [128, D], F32, kind="ExternalInput")
    grow = nc.dram_tensor("grow", [128, D], F32, kind="ExternalInput")
    berow = nc.dram_tensor("berow", [128, D], F32, kind="ExternalInput")
    iota_d = nc.dram_tensor("iota", [128, 128], BF16, kind="ExternalInput")
    slotrow_d = nc.dram_tensor("slotrow", [128, NTIL], BF16, kind="ExternalInput")
    scale_d = nc.dram_tensor("scale", [128, NTIL], F32, kind="ExternalInput")
    idx_dst = nc.dram_tensor("idx_dst", [128, NPOS // 16], I16, kind="ExternalInput")
    idx_src = nc.dram_tensor("idx_src", [128, NPOS // 16], I16, kind="ExternalInput")
    ih_d = nc.dram_tensor("ih", [128, NW * C], F32, kind="ExternalInput")
    curinit = nc.dram_tensor("curinit", [SHP, 128], BF16, kind="ExternalInput")
    out = nc.dram_tensor("out", [SHP, C], F32, kind="ExternalOutput")

    # ---- internal DRAM
    # hncur rows pack [hn (256B) | cur0 (256B)] so ONE 512B gather per edge
    # serves both the phase-B similarity and the layer-0 aggregation.
    hn_c = nc.dram_tensor("hn_c", [SHP, D], BF16)
    hncur_loc = nc.dram_tensor("hncur_loc", [SHP, 256], BF16)
    hncur_full = nc.dram_tensor(
        "hncur_full", [NCORES * SHP, 256], BF16, addr_space="Shared"
    )
    curloc = [nc.dram_tensor(f"curloc{i}", [SHP, 128], BF16) for i in range(2)]
    curfull = [
        nc.dram_tensor(f"curfull{i}", [NCORES * SHP, 128], BF16, addr_space="Shared")
        for i in range(2)
    ]

    rg = [list(range(NCORES))]
    tc = nc._tc

    # =================================================== phase A: features
    with tc.tile_pool(name="pa", bufs=2) as pa, tc.tile_pool(
        name="pa1", bufs=1
    ) as pa1, tc.tile_pool(name="pap", bufs=2, space="PSUM") as pap:
        ft = pa1.tile([128, SHP], F32)
        nc.sync.dma_start(out=ft[:], in_=featT[:])
        wts = pa1.tile([128, D], F32)
        nc.sync.dma_start(out=wts[:], in_=wt[:])
        brs = pa1.tile([128, D], F32)
        nc.sync.dma_start(out=brs[:], in_=brow[:])
        grs = pa1.tile([128, D], F32)
        nc.sync.dma_start(out=grs[:], in_=grow[:])
        bes = pa1.tile([128, D], F32)
        nc.sync.dma_start(out=bes[:], in_=berow[:])
        epsl = pa1.tile([128, 1], F32)
        nc.vector.memset(epsl[:], EPS_LN)

        for t in range(NT):
            ps = pap.tile([128, D], F32)
            nc.tensor.matmul(
                out=ps[:],
                lhsT=ft[:, t * 128 : (t + 1) * 128],
                rhs=wts[:],
                start=True,
                stop=True,
            )
            h = pa.tile([128, D], F32)
            nc.vector.tensor_tensor(
                out=h[:], in0=ps[:], in1=brs[:], op=mybir.AluOpType.add
            )
            stats = pa.tile([128, 6], F32)
            nc.vector.bn_stats(out=stats[:], in_=h[:])
            mv = pa.tile([128, 2], F32)
            nc.vector.bn_aggr(out=mv[:], in_=stats[:])
            std = pa.tile([128, 1], F32)
            nc.scalar.activation(
                out=std[:],
                in_=mv[:, 1:2],
                func=mybir.ActivationFunctionType.Sqrt,
                bias=epsl[:],
                scale=1.0,
            )
            rstd = pa.tile([128, 1], F32)
            nc.vector.reciprocal(out=rstd[:], in_=std[:])
            hc = pa.tile([128, D], F32)
            nc.vector.scalar_tensor_tensor(
                out=hc[:],
                in0=h[:],
                scalar=mv[:, 0:1],
                in1=rstd[:].to_broadcast([128, D]),
                op0=mybir.AluOpType.subtract,
                op1=mybir.AluOpType.mult,
            )
            hg = pa.tile([128, D], F32)
            nc.vector.tensor_tensor(
                out=hg[:], in0=hc[:], in1=grs[:], op=mybir.AluOpType.mult
            )
            hb = pa.tile([128, D], F32)
            nc.vector.tensor_tensor(
                out=hb[:], in0=hg[:], in1=bes[:], op=mybir.AluOpType.add
            )
            hr = pa.tile([128, D], F32)
            nc.vector.tensor_scalar(
                out=hr[:],
                in0=hb[:],
                scalar1=0.0,
                scalar2=None,
                op0=mybir.AluOpType.max,
            )
            sq = pa.tile([128, D], F32)
            nc.vector.tensor_tensor(
                out=sq[:], in0=hr[:], in1=hr[:], op=mybir.AluOpType.mult
            )
            ssum = pa.tile([128, 1], F32)
            nc.vector.tensor_reduce(
                out=ssum[:], in_=sq[:], axis=mybir.AxisListType.X,
                op=mybir.AluOpType.add,
            )
            snrm = pa.tile([128, 1], F32)
            nc.scalar.activation(
                out=snrm[:],
                in_=ssum[:],
                func=mybir.ActivationFunctionType.Sqrt,
            )
            scl = pa.tile([128, 1], F32)
            nc.vector.tensor_scalar(
                out=scl[:],
                in0=snrm[:],
                scalar1=EPS_COS,
                scalar2=None,
                op0=mybir.AluOpType.max,
            )
            rcl = pa.tile([128, 1], F32)
            nc.vector.reciprocal(out=rcl[:], in_=scl[:])
            hnf = pa.tile([128, D], BF16)
            nc.vector.tensor_scalar(
                out=hnf[:],
                in0=hr[:],
                scalar1=rcl[:],
                scalar2=None,
                op0=mybir.AluOpType.mult,
            )
            nc.sync.dma_start(
                out=hn_c[t * 128 : (t + 1) * 128, :], in_=hnf[:]
            )
            nc.sync.dma_start(
                out=hncur_loc[t * 128 : (t + 1) * 128, 0:128], in_=hnf[:]
            )

    nc.sync.dma_start(out=hncur_loc[:, 128:256], in_=curinit[:])
    nc.gpsimd.collective_compute(
        "AllGather",
        mybir.AluOpType.bypass,
        ins=[hncur_loc[:]],
        outs=[hncur_full[:]],
        replica_groups=rg,
    )

    # ------------------------------------------------- resident SBUF state
    pr_cm = tc.tile_pool(name="pr", bufs=1)
    pr = pr_cm.__enter__()
    w_sb = pr.tile([128, NTIL], BF16)
    slotrow = pr.tile([128, NTIL], BF16)
    nc.sync.dma_start(out=slotrow[:], in_=slotrow_d[:])
    iota = pr.tile([128, 128], BF16)
    nc.sync.dma_start(out=iota[:], in_=iota_d[:])
    ih_sb = pr.tile([128, NW * C], F32)
    nc.sync.dma_start(out=ih_sb[:], in_=ih_d[:])
    rdenom = pr.tile([128, NW], F32)
    curtile = pr.tile([128, NW, C + 1], BF16)
    nc.vector.memset(curtile[:], 1.0)  # col C stays 1.0 (ones column)

    # ============================== phase B + layers (B fused into layer 0)
    pcp_cm = tc.tile_pool(name="pcp", bufs=1, space="PSUM")
    pcp = pcp_cm.__enter__()
    acc_ps = pcp.tile([128, NW, ACCW], F32)

    # ------------------------------------------------ layer 0 (+ phase B)
    with tc.tile_pool(name="pg0", bufs=3) as pg, \
         tc.tile_pool(name="poh0", bufs=3) as poh, \
         tc.tile_pool(name="pv0", bufs=4) as pv, \
         tc.tile_pool(name="pb0", bufs=1) as pb1:
        scale_sb = pb1.tile([128, NTIL], F32)
        nc.sync.dma_start(out=scale_sb[:], in_=scale_d[:])
        for ci, (s, p0, np_) in enumerate(chunks):
            til = np_ // 128
            t0 = p0 // 128
            icol = p0 // 16
            qn = ci % 4
            idd = pg.tile([128, CHUNK // 16], I16, tag="idd")
            nc.sync.dma_start(
                out=idd[:, : np_ // 16],
                in_=idx_dst[:, icol : icol + np_ // 16],
            )
            G = pg.tile([128, CHUNK // 128, 256], BF16, tag="G")
            nc.gpsimd.dma_gather(
                out_ap=G[:, :til],
                in_ap=hncur_full[s * SHP : (s + 1) * SHP, :],
                idxs_ap=idd[:, : np_ // 16],
                num_idxs=np_,
                num_idxs_reg=np_,
                elem_size=256,
                single_packet=False,
                queue_num=qn,
            )
            # fused phase B: gather hn[src], w = sig*cos(hn_s, hn_d)
            ids = pg.tile([128, CHUNK // 16], I16, tag="ids")
            nc.sync.dma_start(
                out=ids[:, : np_ // 16],
                in_=idx_src[:, icol : icol + np_ // 16],
            )
            gs = pg.tile([128, CHUNK // 128, D], BF16, tag="gs")
            nc.gpsimd.dma_gather(
                out_ap=gs[:, :til],
                in_ap=hn_c[:],
                idxs_ap=ids[:, : np_ // 16],
                num_idxs=np_,
                num_idxs_reg=np_,
                elem_size=D,
                single_packet=False,
                queue_num=qn,
            )
            nc.vector.tensor_tensor(
                out=gs[:, :til], in0=gs[:, :til], in1=G[:, :til, 0:D],
                op=mybir.AluOpType.mult,
            )
            sim = pv.tile([128, CHUNK // 128, 1], F32, tag="sim")
            nc.vector.tensor_reduce(
                out=sim[:, :til], in_=gs[:, :til],
                axis=mybir.AxisListType.X, op=mybir.AluOpType.add,
            )
            nc.vector.tensor_tensor(
                out=w_sb[:, t0 : t0 + til],
                in0=sim[:, :til, 0],
                in1=scale_sb[:, t0 : t0 + til],
                op=mybir.AluOpType.mult,
            )
            oh = poh.tile([128, CHUNK // 128, 128], BF16, tag="oh")
            nc.vector.tensor_tensor(
                out=oh[:, :til],
                in0=iota[:].unsqueeze(1).to_broadcast([128, til, 128]),
                in1=slotrow[:, t0 : t0 + til]
                .unsqueeze(2)
                .to_broadcast([128, til, 128]),
                op=mybir.AluOpType.is_equal,
            )
            # V = w * (cur ++ ones) on the 17 used columns
            V = pv.tile([128, CHUNK // 128, C + 1], BF16, tag="V")
            nc.vector.tensor_tensor(
                out=V[:, :til],
                in0=G[:, :til, 128 : 128 + C + 1],
                in1=w_sb[:, t0 : t0 + til]
                .unsqueeze(2)
                .to_broadcast([128, til, C + 1]),
                op=mybir.AluOpType.mult,
            )
            for ti in range(til):
                t = t0 + ti
                w = int(tilewin[t])
                nc.tensor.matmul(
                    out=acc_ps[:, w, 0 : C + 1],
                    lhsT=oh[:, ti, :],
                    rhs=V[:, ti, :],
                    start=bool(first[t]),
                    stop=bool(last[t]),
                    skip_group_check=True,
                )

    # layer-0 epilogue: denominators + cur_1
    with tc.tile_pool(name="pe0", bufs=1) as pe:
        tw = pe.tile([128, NW], F32)
        nc.vector.tensor_scalar(
            out=tw[:], in0=acc_ps[:, :, C], scalar1=0.0, scalar2=None,
            op0=mybir.AluOpType.add,
        )
        mask = pe.tile([128, NW], F32)
        nc.vector.tensor_scalar(
            out=mask[:], in0=tw[:], scalar1=0.0, scalar2=None,
            op0=mybir.AluOpType.is_gt,
        )
        ones = pe.tile([128, NW], F32)
        nc.vector.memset(ones[:], 1.0)
        md = pe.tile([128, NW], F32)
        nc.vector.scalar_tensor_tensor(
            out=md[:], in0=mask[:], scalar=-1.0, in1=ones[:],
            op0=mybir.AluOpType.mult, op1=mybir.AluOpType.add,
        )
        tm = pe.tile([128, NW], F32)
        nc.vector.tensor_tensor(
            out=tm[:], in0=tw[:], in1=mask[:], op=mybir.AluOpType.mult
        )
        dn = pe.tile([128, NW], F32)
        nc.vector.tensor_tensor(
            out=dn[:], in0=tm[:], in1=md[:], op=mybir.AluOpType.add
        )
        rec = pe.tile([128, NW], F32)
        nc.vector.reciprocal(out=rec[:], in_=dn[:])
        nc.vector.tensor_scalar(
            out=rdenom[:], in0=rec[:], scalar1=ALPHA, scalar2=None,
            op0=mybir.AluOpType.mult,
        )
        tmp = pe.tile([128, NW, C], F32)
        nc.vector.tensor_tensor(
            out=tmp[:],
            in0=acc_ps[:, :, 0:C],
            in1=rdenom[:].unsqueeze(2).to_broadcast([128, NW, C]),
            op=mybir.AluOpType.mult,
        )
        nc.vector.tensor_tensor(
            out=curtile[:, :, 0:C],
            in0=tmp[:],
            in1=ih_sb[:].rearrange("p (t c) -> p t c", c=C),
            op=mybir.AluOpType.add,
        )
        nc.sync.dma_start(
            out=curloc[1][:, 0 : C + 1].rearrange("(t p) c -> p t c", p=128),
            in_=curtile[:],
        )
    nc.gpsimd.collective_compute(
        "AllGather",
        mybir.AluOpType.bypass,
        ins=[curloc[1][:]],
        outs=[curfull[1][:]],
        replica_groups=rg,
    )

    # layer indices: loaded after layer-0 pools release (SBUF reuse)
    pr2_cm = tc.tile_pool(name="pr2", bufs=1)
    pr2 = pr2_cm.__enter__()
    idx_sb = pr2.tile([128, NPOS // 16], I16)
    nc.sync.dma_start(out=idx_sb[:], in_=idx_dst[:])

    # --------------------------------------------------- layers 1..4
    for layer in range(1, NUM_LAYERS):
        src_tab = curfull[layer % 2]
        with tc.tile_pool(name=f"pg{layer}", bufs=6) as pg, \
             tc.tile_pool(name=f"poh{layer}", bufs=3) as poh, \
             tc.tile_pool(name=f"pv{layer}", bufs=3) as pv:
            for ci, (s, p0, np_) in enumerate(chunks):
                til = np_ // 128
                t0 = p0 // 128
                icol = p0 // 16
                G = pg.tile([128, CHUNK // 128, 128], BF16, tag="G")
                nc.gpsimd.dma_gather(
                    out_ap=G[:, :til],
                    in_ap=src_tab[s * SHP : (s + 1) * SHP, :],
                    idxs_ap=idx_sb[:, icol : icol + np_ // 16],
                    num_idxs=np_,
                    num_idxs_reg=np_,
                    elem_size=128,
                    single_packet=False,
                    queue_num=ci % 4,
                )
                oh = poh.tile([128, CHUNK // 128, 128], BF16, tag="oh")
                nc.vector.tensor_tensor(
                    out=oh[:, :til],
                    in0=iota[:].unsqueeze(1).to_broadcast([128, til, 128]),
                    in1=slotrow[:, t0 : t0 + til]
                    .unsqueeze(2)
                    .to_broadcast([128, til, 128]),
                    op=mybir.AluOpType.is_equal,
                )
                V = pv.tile([128, CHUNK // 128, C], BF16, tag="V")
                nc.vector.tensor_tensor(
                    out=V[:, :til],
                    in0=G[:, :til, 0:C],
                    in1=w_sb[:, t0 : t0 + til]
                    .unsqueeze(2)
                    .to_broadcast([128, til, C]),
                    op=mybir.AluOpType.mult,
                )
                for ti in range(til):
                    t = t0 + ti
                    w = int(tilewin[t])
                    nc.tensor.matmul(
                        out=acc_ps[:, w, 0:C],
                        lhsT=oh[:, ti, :],
                        rhs=V[:, ti, :],
                        start=bool(first[t]),
                        stop=bool(last[t]),
                        skip_group_check=True,
                    )

        # ------------------------------------------------- layer epilogue
        with tc.tile_pool(name=f"pe{layer}", bufs=1) as pe:
            tmp = pe.tile([128, NW, C], F32)
            nc.vector.tensor_tensor(
                out=tmp[:],
                in0=acc_ps[:, :, 0:C],
                in1=rdenom[:].unsqueeze(2).to_broadcast([128, NW, C]),
                op=mybir.AluOpType.mult,
            )
            if layer < NUM_LAYERS - 1:
                nc.vector.tensor_tensor(
                    out=curtile[:, :, 0:C],
                    in0=tmp[:],
                    in1=ih_sb[:].rearrange("p (t c) -> p t c", c=C),
                    op=mybir.AluOpType.add,
                )
                nc.sync.dma_start(
                    out=curloc[(layer + 1) % 2][:, 0 : C + 1].rearrange(
                        "(t p) c -> p t c", p=128
                    ),
                    in_=curtile[:],
                )
                nc.gpsimd.collective_compute(
                    "AllGather",
                    mybir.AluOpType.bypass,
                    ins=[curloc[(layer + 1) % 2][:]],
                    outs=[curfull[(layer + 1) % 2][:]],
                    replica_groups=rg,
                )
            else:
                cur = pe.tile([128, NW, C], F32)
                nc.vector.tensor_tensor(
                    out=cur[:],
                    in0=tmp[:],
                    in1=ih_sb[:].rearrange("p (t c) -> p t c", c=C),
                    op=mybir.AluOpType.add,
                )
                nc.sync.dma_start(
                    out=out[:].rearrange("(t p) c -> p t c", p=128), in_=cur[:]
                )

    pcp_cm.__exit__(None, None, None)
    pr2_cm.__exit__(None, None, None)
    pr_cm.__exit__(None, None, None)


def fix_queue_nums(nc):
    """Align each custom Pool-DMA's SWDGE queue with its Tile-assigned
    DMASW sem lane (shadow-sem rule: one queue per sem)."""
    n = 0
    for inst in nc.inst_map.values():
        if type(inst).__name__ in ("InstDMAGatherAnt", "InstDMAScatterAddAnt"):
            p = getattr(inst, "bass_scheduled_proc", None)
            if p is not None and 11 <= p <= 18:
                inst.queue_num = (p - 11) % 4
                n += 1
    return n


# ------------------------------------------------------------------ runtime

_CACHE = {}


def _get_nc(cfg):
    key = tuple(cfg)
    if key not in _CACHE:
        nc = bacc.Bacc(
            "TRN2",
            target_bir_lowering=False,
            debug=False,
            enable_asserts=False,
            num_devices=NCORES,
            num_swdge_queues=4,
        )
        with tile.TileContext(nc) as tc:
            nc._tc = tc
            build(nc, cfg)
        fix_queue_nums(nc)
        nc.compile()
        _CACHE[key] = nc
    return _CACHE[key]


TRACE = False
LAST_RESULT = None


def _install_ntff_hook():
    """Provide antenv.axon_hooks (absent in this image) so that
    run_bass_kernel_spmd(trace=True) can capture NTFF profiles."""
    import types

    if "antenv.axon_hooks" in sys.modules:
        return
    import antenv

    mod = types.ModuleType("antenv.axon_hooks")
    mod._hook = None

    def set_axon_ntff_profile_hook(h):
        mod._hook = h

    def get_axon_ntff_profile_hook():
        return mod._hook

    mod.set_axon_ntff_profile_hook = set_axon_ntff_profile_hook
    mod.get_axon_ntff_profile_hook = get_axon_ntff_profile_hook
    sys.modules["antenv.axon_hooks"] = mod
    antenv.axon_hooks = mod
    try:
        from trn_agent_boot.trn_boot import _ntff_profile_via_ctypes

        h = _ntff_profile_via_ctypes("/opt/axon/libaxon_pjrt.so")
        if h is not None:
            set_axon_ntff_profile_hook(h)
    except Exception as e:  # degrade to no tracing
        print(f"ntff hook install failed: {e}", file=sys.stderr)


def kernel(**inputs):
    global LAST_RESULT
    if TRACE:
        _install_ntff_hook()
    in_maps, cfg = preprocess(inputs)
    nc = _get_nc(cfg)
    res = run_bass_kernel_spmd(
        nc, in_maps, core_ids=list(range(NCORES)), trace=TRACE
    )
    LAST_RESULT = res
    full = np.zeros((N, C), np.float32)
    for c in range(NCORES):
        full[c * SH : (c + 1) * SH] = res.results[c]["out"][:SH]
    return full
